# revision 37
# baseline (speedup 1.0000x reference)
"""Causal self-attention (B=2, T=2048, C=1024, H=16) on 8 trn2 NeuronCores.

Sharding: batch x head-group. Core c handles batch b = c//4 and heads
[4*(c%4), 4*(c%4)+4). Each core computes qkv for its head slice, causal
attention, and a partial c_proj ([T, C] over its 256 input rows of W_proj);
the host sums the 4 bf16 partials per batch (data-parallel over b,
tensor-parallel over heads with the all-reduce done on host).  ~183-186us
measured on HW (run-to-run HAM-phase variance is +/-8us; a ~215us
predecessor of this schedule, ~227us before that), rel err ~3.8e-3 (bf16
operands/output; fp32 accumulate everywhere).

Device dataflow (per core):
  - qT, kT computed in [D', T] layout (D' = 256 local head dims), v in [T, D']
    layout, all from host-pre-transposed bf16 xT [C, T].
  - attention per head: S^T[k, q] = kT.T-slice @ qT-slice so that softmax's
    key dim is the PSUM partition dim; the padding mask becomes a
    per-partition bias of the exp activation.  exp(S/8 + maskNEG) -> U^T.
    Causal masking = one [128,128] upper-tri elementwise multiply per
    diagonal tile.  O^T[d, q] accumulates lhsT=v_aug [k, 128] (cols 64:96
    zero, col 96 = 1.0 so psum row 96 collects the softmax denominator for
    free; cols 97:128 zero-padded so the ldweights gets Fast Weight Load),
    rhs=U^T.
  - normalize: denominator [1, W] -> DMA-reshape to [64,16] -> reciprocal ->
    DMA broadcast -> multiply = yT rows, exactly the lhsT of c_proj; the two
    tail sub-blocks instead use a DMA-free chain (approx reciprocal on the
    raw row + Pool partition-broadcast), split into 256+512 column sub-chains
    for the 768 block so the proj tiles 8,9 (which read only q 1024:1280)
    unblock ~2.5us before the full chain.  NOTE: reciprocal_approx_fast
    reading PSUM directly passes CoreSim but returns garbage on HW — the dn
    staging copy through SBUF is required.

Schedule notes.  The PE clock gate (HAM) is the whole game: K drops 8/8 ->
4/8 (half clock) whenever a free-running 3.4us window sees more than ~0.8us
of PE idle, and once cold at saturated demand the re-warm is sticky — a
97%-busy stream can stay at half clock for 20us+ (a 27us cold window cost
~13us of wall in one trace).  The fix is not filler volume but ELIMINATING
EVERY >=0.8us PE gap: with all block-transition stalls removed, K=8/8 held
for 143us straight and k=4 time fell 95us -> 19us (215.6 -> 183.6us wall).
Engine placement of the tiny glue ops is what creates or removes those
gaps, because every engine queue is strict FIFO:
  - the softmax-denominator extraction (dn copy, [1,W] psum row 96 -> sbuf)
    rides ACT, which idles ~5us at every block transition (its next exp is
    S-gated); on DVE it held the braid casts/bias-adds — and with them the
    pool_x banks the next block's first S matmuls reuse — behind a ~1.2us
    copy, and each such gap tripped a HAM re-throttle;
  - proj psum->sbuf casts ride DVE except the middle tile of each 3-tile
    braid (1, 4) and the last tile (15), which ride ACT so the serialized
    DVE cast chain never gates the next block's pool_x rotation;
  - full-block normalizes are EMITTED late (defer_norm) — after the
    following braid's DVE ops — so the reciprocal/multiply sit behind, not
    ahead of, the bank-gating braid work in the DVE FIFO;
  - qk braids run all-psq-then-all-psk (accumulation closes at burst mid),
    and v braids run ts-outer, so the braid's first matmuls use banks freed
    long ago and the bank-gating bias-adds finish by burst end;
  - the full-block normalize's 4-DMA reshape chain rides the gpsimd queue,
    NOT sync: behind sync's 512KB xts transfers its final DVE multiply
    landed 10us+ late and then blocked the DVE FIFO (and the next block's S
    matmuls) mid-braid — moving the queue measured -13us in-epoch (199->186)
    and k=4 time held at ~20us with a 147us continuous K=8/8 stretch;
Things measured and rejected: kT bias-adds on ACT Identity + proj pairs
braided inside the tail256 block (together +44us — do not revisit blindly);
proj tile pairs alone (+3us, but kept: with the dn-on-ACT split they
measure net positive as part of the endgame).
  - input DMAs are batched (one trigger per tensor / half-tblock): a DMA
    trigger costs ~650ns of queue issue time, and per-chunk DMAs would
    serialize the kernel start for ~20us;
  - x tblock-0 DMAs issue before the weight DMAs (they gate the first
    matmuls); a minimal fp32 junk-matmul burst covers the short remaining
    window so real work starts at full clock without charging much of the
    activity budget;
  - qkv t>=1024 tblocks are braided between attention q<1024 heads — split
    into ~4.3us qk and ~5us v bursts in four separate gaps (a whole 8.3us
    tblock burst reliably trips the activity throttler; halving the burst
    size bought a continuous ~99us full-clock window through m0-late + m1
    and ~10us of wall).  c_proj t<1024 tiles go between attention q>=1024
    heads.  The PE is strict FIFO: filler inside a block would delay the
    exp-paced S/O chain, and a proj tile placed right after the block
    producing its yT stalls on that block's normalize latency — so bursts
    go between blocks, one block late;
  - the junk pre-warm is net-positive even with batched DMAs (removing it
    measured +3us: real work starting at the cold clock costs more than
    the junk's activity-budget charge);
  - attention emits S(j+1) before O(j) (engine instruction order is static);
  - heads run 2,3,0,1; c_proj accumulates W_proj chunk 1 (heads 2,3, which
    finish first) before chunk 0;
  - the last head's q>=1024 block is split 768+256 wide: proj tiles 8..13
    run between the sub-blocks (8,9 as a chunk-1-first pair hiding the 768
    normalize) and only the 14,15 pair plus a short DMA-free normalize
    trail the final exp;
  - bulk output DMAs ride the gpsimd queue; the last tiles alternate onto
    the by-then-idle Sync queue so the final drain overlaps.
"""

import contextlib
import functools
import sys

sys.path.insert(0, "/opt/trn_rl_repo")

import numpy as np

import concourse.bacc as bacc
import concourse.mybir as mybir
import concourse.tile as tile
from concourse import bass_utils
from concourse.alu_op_type import AluOpType

B, T, C, H, D = 2, 2048, 1024, 16, 64
NEG = -1e10
NCORES = 8
HEADS_PER_CORE = 4
DLOC = HEADS_PER_CORE * D  # 256 local head dims per core
F32 = mybir.dt.float32
F32R = mybir.dt.float32r
BF16 = mybir.dt.bfloat16
AF = mybir.ActivationFunctionType

# bf16 for the qkv input matmuls (x, W_attn slices) and the c_proj matmul
# (yT, W_proj): halves the input-DMA window that gates the kernel start.
# S^T / O^T attention matmuls stay fp32r.
USE_BF16_INPUTS = True
IN_DT = BF16 if USE_BF16_INPUTS else F32R
# ~duration of junk pre-warm matmuls covering the input-DMA window (ns).
# Input DMAs are batched (one trigger per tensor / half-tblock), so the
# window is short; junk matmuls charge the PE activity monitor's utilization
# budget, so emit as few as possible.
WARM_NS = 8300

NTB = T // 512  # 4 t-blocks in qkv phase
NKC = T // 128  # 16 k-chunks
NQB = 2  # attention q-blocks of 1024


def _pieces(a, end=1024):
    """Split [a, end) at 512-boundaries (psum bank boundaries)."""
    cuts = [a]
    b = (a // 512 + 1) * 512
    while b < end:
        cuts.append(b)
        b += 512
    cuts.append(end)
    return list(zip(cuts[:-1], cuts[1:]))


class Ctx:
    pass


def _emit_prewarm(nc, g):
    """Dependency-free fp32 matmuls sized to cover the initial input-DMA
    window: the PE warms up on junk (instead of idling cold) and hands off
    at full clock to a fully-fed dense qkv stream.  Cold-busy is what the
    HAM punishes; cold-idle under DMA is free, but a ragged DMA-paced start
    re-throttles the clock over and over."""
    # fp32 N=512 matmul = 4 cyc/col: ~1.7us cold, ~0.85us warm; warm-up
    # transition after ~3.4us
    n = 3 + max(0, int((WARM_NS - 5200) / 880))
    ps = g.pool_x.tile([128, 1024], F32, tag="px", name="warm_ps")
    for i in range(n):
        nc.tensor.matmul(
            ps[:, 0:512],
            g.warm_sb[:, 0:128],
            g.warm_sb,
            start=(i == 0),
            stop=(i == n - 1),
        )
    wsink = g.rnpool.tile([1, 128], F32, tag="wsink", name="wsink")
    nc.vector.tensor_copy(wsink, ps[0:1, 0:128])
    nc.sync.dma_start(out=g.rn_dram.ap()[0:1, 0:128], in_=wsink)


def _dma_xts(nc, g, tb):
    """Issue the batched x DMAs for one tblock (two half-tblock triggers)."""
    xts = g.xpool.tile([128, 8, 512], IN_DT, tag="xts", name="xts")
    nc.sync.dma_start(out=xts[:, 0:4, :], in_=g.xT_r[:, 0:4, tb * 512 : (tb + 1) * 512])
    nc.sync.dma_start(out=xts[:, 4:8, :], in_=g.xT_r[:, 4:8, tb * 512 : (tb + 1) * 512])
    return xts


def _emit_qkv_qk(nc, g, tb, xts):
    """q/k projections for t in [tb*512, (tb+1)*512) (~4.3us PE burst)."""
    psq = g.pool_x.tile([128, 1024], F32, tag="px", name="psq")
    psk = g.pool_x.tile([128, 1024], F32, tag="px", name="psk")
    # all-psq-then-all-psk (not interleaved): psq's accumulation closes at the
    # burst midpoint, so its DVE bias-adds overlap the psk half and psq's
    # banks (which the NEXT attention block's first S matmul reuses via the
    # pool_x rotation) are free by burst end instead of ~2us after
    for ps, w in ((psq, g.wq_sb), (psk, g.wk_sb)):
        for cc in range(8):
            for dt_ in range(2):
                nc.tensor.matmul(
                    ps[:, dt_ * 512 : (dt_ + 1) * 512],
                    w[:, cc, dt_ * 128 : (dt_ + 1) * 128],
                    xts[:, cc, :],
                    start=cc == 0,
                    stop=cc == 7,
                )
    # qT adds on DVE (hidden under the psk half-burst).  kT adds gate the
    # next attention block's second S matmul via the pool_x bank rotation
    # and can only start at burst end, so they're SPLIT across engines —
    # dt0 on ACT Identity (same table set as Exp, no ACT_TABLE_LOAD), dt1
    # on DVE — freeing psk's banks in ~0.75us instead of 1.5us serial.
    for dt_ in range(2):
        nc.vector.tensor_scalar(
            out=g.qT_sb[:, dt_, tb * 512 : (tb + 1) * 512],
            in0=psq[:, dt_ * 512 : (dt_ + 1) * 512],
            scalar1=g.bq_sb[:, dt_ : dt_ + 1],
            scalar2=None,
            op0=AluOpType.add,
        )
    nc.scalar.activation(
        out=g.kT_sb[:, 0, tb * 512 : (tb + 1) * 512],
        in_=psk[:, 0:512],
        func=AF.Identity,
        bias=g.bk_sb[:, 0:1],
        scale=1.0,
    )
    nc.vector.tensor_scalar(
        out=g.kT_sb[:, 1, tb * 512 : (tb + 1) * 512],
        in0=psk[:, 512:1024],
        scalar1=g.bk_sb[:, 1:2],
        scalar2=None,
        op0=AluOpType.add,
    )


def _emit_qkv_v(nc, g, tb, xts):
    """v projection for t in [tb*512, (tb+1)*512) (~5us PE burst)."""
    # four concurrent v chains need four distinct psum banks (an accumulation
    # group's start=True zeroes its whole 2KB bank): chain ts lives in tile
    # ts//2 at column offset (ts%2)*512
    psv = [
        g.pool_o.tile([128, 1024], F32, tag="pso", name="psvA"),
        g.pool_o.tile([128, 1024], F32, tag="pso", name="psvB"),
    ]

    def vslice(ts, width=256):
        return psv[ts // 2][:, (ts % 2) * 512 : (ts % 2) * 512 + width]

    # ts-outer: the psvA chains (ts 0,1) complete before psvB's first matmul,
    # so the ~2us wait for the preceding attention block's normalize to free
    # pool_o buffer B hides under real PE work instead of stalling the whole
    # burst (PE is strict FIFO)
    for ts in range(4):
        for cc in range(8):
            nc.tensor.matmul(
                vslice(ts),
                xts[:, cc, ts * 128 : (ts + 1) * 128],
                g.wv_sb[:, cc, :],
                start=cc == 0,
                stop=cc == 7,
            )
    for ts in range(4):
        kc = tb * 4 + ts
        nc.vector.tensor_tensor(
            out=g.vaug[:, kc, :, 0:D],
            in0=vslice(ts).rearrange("p (h d) -> p h d", h=4),
            in1=g.bvb_sb.rearrange("p (h d) -> p h d", h=4),
            op=AluOpType.add,
        )


def _emit_qkv_tblock(nc, g, tb, xts=None):
    """Full qkv for one tblock (front, non-braided tblocks)."""
    if xts is None:
        xts = _dma_xts(nc, g, tb)
    _emit_qkv_qk(nc, g, tb, xts)
    _emit_qkv_v(nc, g, tb, xts)


def _emit_attention_block(nc, g, h, q0, width, braid=(), defer_norm=False):
    """One head x one [q0, q0+width) q-block of causal attention.

    braid: iterable of (after_j, fn) — fn() is emitted right after emit_O of
    step after_j.  Braided proj tiles must come in PAIRS of pool_x
    allocations so the pss double-buffer parity is preserved: the second
    psp of a pair naturally stalls on the in-flight exp, self-pacing the
    filler work to the exp stream instead of bursting.
    """
    prow = (h % 2) * 64
    pi = h // 2
    njs = (q0 + width) // 128
    last_bank0 = q0 // 128 + 3 if width > 512 else njs - 1
    pso = g.pool_o.tile([128, 1024], F32, tag="pso", name="pso")
    uts = {}
    braid = dict(braid)

    def emit_S_exp(j):
        # S^T then exp; the O^T consuming exp(j) is emitted after S(j+1) so
        # the PE's static instruction order never waits on the ACT engine
        a = max(0, 128 * j - q0)
        pss = g.pool_x.tile([128, 1024], F32, tag="px", name="pss")
        for c0, c1 in _pieces(a, width):
            nc.tensor.matmul(
                pss[:, c0:c1],
                g.kT_sb[prow : prow + 64, pi, j * 128 : (j + 1) * 128],
                g.qT_sb[prow : prow + 64, pi, q0 + c0 : q0 + c1],
                start=True,
                stop=True,
            )
        ut = g.utpool.tile([128, 1024], BF16, tag="ut", name="ut")
        uts[j] = ut
        nc.scalar.activation(
            out=ut[:, a:width],
            in_=pss[:, a:width],
            func=AF.Exp,
            bias=g.mneg_sb[:, j : j + 1],
            scale=0.125,
        )
        if 128 * j >= q0:
            nc.vector.tensor_mul(ut[:, a : a + 128], ut[:, a : a + 128], g.tri_sb)

    def emit_O(j):
        a = max(0, 128 * j - q0)
        ut = uts.pop(j)
        for c0, c1 in _pieces(a, width):
            stop = j == (last_bank0 if c0 < 512 else njs - 1)
            nc.tensor.matmul(
                pso[:, c0:c1],
                g.vaug[:, j, h, :],
                ut[:, c0:c1],
                start=(j == 0),
                stop=stop,
            )

    def emit_norm():
        # normalize: yT rows = O^T * (1/denom) broadcast.  The denominator
        # row is [1, width]; reciprocal there runs on one DVE lane, so for
        # full blocks it is DMA-reshaped to [64, 16] first (chain DMAs ride
        # the otherwise-idle Sync queue).  The tail sub-blocks use a
        # DMA-free chain instead (approx reciprocal on the row + Pool
        # partition-broadcast): ~5us shorter latency where it is exposed.
        if width == 1024:
            hm = h * NQB + q0 // 1024
            dn = g.rnpool.tile([1, 1024], F32, tag="dn", name="dn")
            # scalar (ACT) on purpose: ACT idles ~5us at every block
            # transition (its next exp is S-gated), while DVE's strict FIFO
            # must not hold the braid's psum->sbuf casts (which gate pool_x
            # bank reuse for the next matmuls) behind this copy
            nc.scalar.copy(dn, pso[96:97, :])
            # the chain DMAs ride the gpsimd queue, NOT sync: sync carries
            # the 512KB xts transfers, and queueing behind one delays this
            # chain's final DVE multiply by 10us+, which then blocks the DVE
            # FIFO (and the next block's S matmuls) when it finally lands
            nc.gpsimd.dma_start(out=g.rn_dram.ap()[hm : hm + 1, :], in_=dn)
            dn_rs = g.rnpool.tile([64, 16], F32, tag="dn_rs", name="dn_rs")
            nc.gpsimd.dma_start(
                out=dn_rs, in_=g.rn_dram.ap()[hm, :].rearrange("(p f) -> p f", p=64)
            )
            rr = g.rnpool.tile([64, 16], F32, tag="rr", name="rr")
            nc.vector.reciprocal(rr, dn_rs)
            nc.gpsimd.dma_start(
                out=g.rn2_dram.ap()[hm, :].rearrange("(p f) -> p f", p=64), in_=rr
            )
            rnb = g.rnpool.tile([64, 1024], F32, tag="rnb", name="rnb")
            nc.gpsimd.dma_start(
                out=rnb, in_=g.rn2_dram.ap()[hm : hm + 1, :].partition_broadcast(64)
            )
            nc.vector.tensor_tensor(
                out=g.yT_sb[prow : prow + 64, pi, q0 : q0 + 1024],
                in0=pso[0:D, :],
                in1=rnb,
                op=AluOpType.mult,
            )
        else:
            # dn staging copy on the by-then-idle ACT (custom-DVE reciprocal
            # reading PSUM directly passes CoreSim but returns garbage on HW).
            # For the 768 block the chain runs in 256+512 column sub-chains:
            # the first 256 q-columns are exactly what the following proj
            # tiles 8,9 read, so they unblock ~2.5us before the full chain
            dn = g.rnpool.tile([1, 1024], F32, tag="dn", name="dn")
            cuts = [(0, 256), (256, width)] if width == 768 else [(0, width)]
            for c0, c1 in cuts:
                nc.scalar.copy(dn[:, c0:c1], pso[96:97, c0:c1])
            for c0, c1 in cuts:
                rrr = g.rnpool.tile([1, 1024], F32, tag="rrr", name="rrr")
                nc.vector.reciprocal_approx_fast(rrr[:, c0:c1], dn[:, c0:c1])
                rnb = g.rnpool.tile([64, 1024], F32, tag="rnb", name="rnb")
                nc.gpsimd.partition_broadcast(rnb[:, c0:c1], rrr[:, c0:c1])
                nc.vector.tensor_tensor(
                    out=g.yT_sb[prow : prow + 64, pi, q0 + c0 : q0 + c1],
                    in0=pso[0:D, c0:c1],
                    in1=rnb[:, c0:c1],
                    op=AluOpType.mult,
                )

    emit_S_exp(0)
    for j in range(1, njs):
        emit_S_exp(j)
        emit_O(j - 1)
        if j - 1 in braid:
            braid[j - 1]()
    emit_O(njs - 1)
    if defer_norm:
        # caller emits the normalize after the following braid tile's DVE
        # work is queued: the dn copy waits on this block's last O, and the
        # DVE FIFO would otherwise hold the braid's psum->sbuf casts (and
        # with them the pool_x banks the next S matmuls need) behind it
        return emit_norm
    emit_norm()


def _emit_attention_tail256(nc, g, h, q0):
    """Final 256-wide sub-block with pair-merged exps.

    This block is exp-instruction-overhead bound (16 tiny exps in the plain
    path).  S(2p) opens a psum bank (start=True zeroes the whole 2KB bank)
    at cols 0:256 and S(2p+1) writes cols 256:512 of the SAME bank with
    start=False — an untouched region's accumulate-bits are clear, so the
    write lands as a plain overwrite — letting ONE exp cover both j's.
    The last two j's (diagonal tri / causal-clipped) stay on the plain path.
    """
    prow = (h % 2) * 64
    pi = h // 2
    width = 256
    njs = (q0 + width) // 128
    pso = g.pool_o.tile([128, 1024], F32, tag="pso", name="pso")
    uts = {}

    def emit_S_pair(p):
        j = 2 * p
        pss = g.pool_x.tile([128, 1024], F32, tag="px", name="pss")
        for jj, (c0, c1), st in ((j, (0, 256), True), (j + 1, (256, 512), False)):
            nc.tensor.matmul(
                pss[:, c0:c1],
                g.kT_sb[prow : prow + 64, pi, jj * 128 : (jj + 1) * 128],
                g.qT_sb[prow : prow + 64, pi, q0 : q0 + width],
                start=st,
                stop=True,
                skip_group_check=not st,
            )
        ut = g.utpool.tile([128, 1024], BF16, tag="ut", name="ut")
        uts[j] = ut
        nc.scalar.activation(
            out=ut[:, 0:512],
            in_=pss[:, 0:512],
            func=AF.Exp,
            bias=g.mneg_sb[:, j : j + 1],
            scale=0.125,
        )

    def emit_O_pair(p):
        j = 2 * p
        ut = uts.pop(j)
        for jj, (c0, c1) in ((j, (0, 256)), (j + 1, (256, 512))):
            nc.tensor.matmul(
                pso[:, 0:256],
                g.vaug[:, jj, h, :],
                ut[:, c0:c1],
                start=(jj == 0),
                stop=False,
            )

    def emit_S_exp_plain(j):
        a = max(0, 128 * j - q0)
        pss = g.pool_x.tile([128, 1024], F32, tag="px", name="pss")
        nc.tensor.matmul(
            pss[:, a:width],
            g.kT_sb[prow : prow + 64, pi, j * 128 : (j + 1) * 128],
            g.qT_sb[prow : prow + 64, pi, q0 + a : q0 + width],
            start=True,
            stop=True,
        )
        ut = g.utpool.tile([128, 1024], BF16, tag="ut", name="ut")
        uts[j] = ut
        nc.scalar.activation(
            out=ut[:, a:width],
            in_=pss[:, a:width],
            func=AF.Exp,
            bias=g.mneg_sb[:, j : j + 1],
            scale=0.125,
        )
        if 128 * j >= q0:
            nc.vector.tensor_mul(ut[:, a : a + 128], ut[:, a : a + 128], g.tri_sb)

    def emit_O_plain(j):
        a = max(0, 128 * j - q0)
        ut = uts.pop(j)
        nc.tensor.matmul(
            pso[:, a:width],
            g.vaug[:, j, h, :],
            ut[:, a:width],
            start=False,
            stop=(j == njs - 1),
        )

    npair = (njs - 2) // 2
    emit_S_pair(0)
    for p in range(1, npair):
        emit_S_pair(p)
        emit_O_pair(p - 1)
    emit_S_exp_plain(njs - 2)
    emit_O_pair(npair - 1)
    emit_S_exp_plain(njs - 1)
    emit_O_plain(njs - 2)
    emit_O_plain(njs - 1)

    # normalize (DMA-free tail chain; dn copy on the by-then-idle ACT)
    dn = g.rnpool.tile([1, 1024], F32, tag="dn", name="dn")
    nc.scalar.copy(dn[:, 0:width], pso[96:97, 0:width])
    rrr = g.rnpool.tile([1, 1024], F32, tag="rrr", name="rrr")
    nc.vector.reciprocal_approx_fast(rrr[:, 0:width], dn[:, 0:width])
    rnb = g.rnpool.tile([64, 1024], F32, tag="rnb", name="rnb")
    nc.gpsimd.partition_broadcast(rnb[:, 0:width], rrr[:, 0:width])
    nc.vector.tensor_tensor(
        out=g.yT_sb[prow : prow + 64, pi, q0 : q0 + width],
        in0=pso[0:D, 0:width],
        in1=rnb[:, 0:width],
        op=AluOpType.mult,
    )


def _emit_proj_pair(nc, g, i0, i1, out, fill_n=0):
    """Two c_proj tiles with BOTH chunk-1 matmul groups emitted before either
    chunk-0 group: chunk 0 reads the just-normalized head-0/1 yT, and the PE
    is strict FIFO, so a stalled chunk-0 matmul would also block the second
    tile's (dependency-free) chunk-1 work behind it.  ~1.7us of chunk-1 work
    hides most of the preceding block's normalize-chain latency.

    fill_n > 0 additionally inserts that many dependency-free fp32 junk
    matmuls (~0.85us each, prewarm-style, into pool_o's free buffer) between
    the chunk-1 and chunk-0 groups: at the 768-block seam the remaining
    normalize-chain latency cannot be covered by real work (PSUM banks), and
    an idle window there trips the HAM into a ~7us half-clock stretch."""
    psps = []
    for i in (i0, i1):
        psp = g.pool_x.tile([128, 1024], F32, tag="px", name="psp")
        psps.append(psp)
        for c0, c1 in _pieces(0):
            nc.tensor.matmul(
                psp[:, c0:c1],
                g.yT_sb[:, 1, i * 128 : (i + 1) * 128],
                g.wp_sb[:, 1, c0:c1],
                start=True,
                stop=False,
            )
    if fill_n:
        jps = g.pool_o.tile([128, 1024], F32, tag="pso", name="seam_junk")
        for i in range(fill_n):
            nc.tensor.matmul(
                jps[:, 0:512],
                g.warm_sb[:, 0:128],
                g.warm_sb,
                start=(i == 0),
                stop=(i == fill_n - 1),
            )
        jsink = g.rnpool.tile([1, 128], F32, tag="wsink", name="jsink")
        nc.vector.tensor_copy(jsink, jps[0:1, 0:128])
    for i, psp in zip((i0, i1), psps):
        for c0, c1 in _pieces(0):
            nc.tensor.matmul(
                psp[:, c0:c1],
                g.yT_sb[:, 0, i * 128 : (i + 1) * 128],
                g.wp_sb[:, 0, c0:c1],
                start=False,
                stop=True,
            )
    for i, psp in zip((i0, i1), psps):
        _emit_proj_out(nc, g, i, psp, out)


def _emit_proj_tile(nc, g, i, out):
    """One plain [128, C] c_proj tile (both chunks ready)."""
    psp = g.pool_x.tile([128, 1024], F32, tag="px", name="psp")
    for step, ic in enumerate((1, 0)):
        for c0, c1 in _pieces(0):
            nc.tensor.matmul(
                psp[:, c0:c1],
                g.yT_sb[:, ic, i * 128 : (i + 1) * 128],
                g.wp_sb[:, ic, c0:c1],
                start=(step == 0),
                stop=(step == 1),
            )
    _emit_proj_out(nc, g, i, psp, out)


def _emit_proj_out(nc, g, i, psp, out):
    ob = g.outp.tile([128, C], BF16, tag="ob_plain", name="ob_plain", bufs=3)
    # casts split DVE/ACT: the middle tile of each 3-tile braid (1, 4) rides
    # the ACT's ~5us block-transition idle window so the DVE cast chain never
    # gates the next block's pool_x bank reuse; tile 15 overlaps tile 14's
    # DVE cast on the by-then-idle ACT
    if i in (1, 4, 15):
        nc.scalar.copy(ob, psp)
    else:
        nc.vector.tensor_copy(ob, psp)
    # tail tiles alternate output DMAs across the gpsimd and (by then idle)
    # sync queues so the final drain overlaps
    if i >= 10 and i % 2 == 1:
        nc.sync.dma_start(out=out.ap()[i * 128 : (i + 1) * 128, :], in_=ob)
    else:
        nc.gpsimd.dma_start(out=out.ap()[i * 128 : (i + 1) * 128, :], in_=ob)


def _build(ctx, nc, tc, ins, out, rn_dram, rn2_dram):
    g = Ctx()
    g.rn_dram, g.rn2_dram = rn_dram, rn2_dram

    singles = ctx.enter_context(tc.tile_pool(name="singles", bufs=1))
    g.pool_x = ctx.enter_context(tc.tile_pool(name="pool_x", bufs=2, space="PSUM"))
    g.pool_o = ctx.enter_context(tc.tile_pool(name="pool_o", bufs=2, space="PSUM"))
    g.xpool = ctx.enter_context(tc.tile_pool(name="xpool", bufs=2))
    g.utpool = ctx.enter_context(tc.tile_pool(name="utpool", bufs=6))
    g.rnpool = ctx.enter_context(tc.tile_pool(name="rnpool", bufs=2))
    g.outp = ctx.enter_context(tc.tile_pool(name="outp", bufs=1))

    # tri mask first: the pre-warm burst depends only on it
    g.tri_sb = singles.tile([128, 128], BF16, name="tri_sb")
    nc.sync.dma_start(out=g.tri_sb, in_=ins["tri"].ap())
    g.warm_sb = singles.tile([128, 512], F32, name="warm_sb")
    nc.vector.memset(g.warm_sb, 0.5)
    _emit_prewarm(nc, g)

    # x tblock 0 DMAs first (they gate the first real matmuls), then weights
    g.xT_r = ins["xT"].ap().rearrange("(c p) t -> p c t", p=128)
    xts0 = _dma_xts(nc, g, 0)

    # --- resident weights / constants ------------------------------------
    # one batched DMA per weight tensor (trigger issue time dominates many
    # small DMAs)
    g.wq_sb = singles.tile([128, 8, DLOC], IN_DT, name="wq_sb")
    g.wk_sb = singles.tile([128, 8, DLOC], IN_DT, name="wk_sb")
    g.wv_sb = singles.tile([128, 8, DLOC], IN_DT, name="wv_sb")
    nc.sync.dma_start(out=g.wq_sb, in_=ins["wq"].ap().rearrange("(c p) m -> p c m", p=128))
    nc.sync.dma_start(out=g.wk_sb, in_=ins["wk"].ap().rearrange("(c p) m -> p c m", p=128))
    nc.sync.dma_start(out=g.wv_sb, in_=ins["wv"].ap().rearrange("(c p) m -> p c m", p=128))

    g.bq_sb = singles.tile([128, 2], F32, name="bq_sb")
    g.bk_sb = singles.tile([128, 2], F32, name="bk_sb")
    nc.sync.dma_start(out=g.bq_sb, in_=ins["bq"].ap().rearrange("i p -> p i"))
    nc.sync.dma_start(out=g.bk_sb, in_=ins["bk"].ap().rearrange("i p -> p i"))
    g.bvb_sb = singles.tile([128, DLOC], F32, name="bvb_sb")
    nc.sync.dma_start(out=g.bvb_sb, in_=ins["bv"].ap().partition_broadcast(128))
    g.mneg_sb = singles.tile([128, NKC], F32, name="mneg_sb")
    nc.sync.dma_start(out=g.mneg_sb, in_=ins["mneg"].ap())

    # --- persistent activations -----------------------------------------
    g.qT_sb = singles.tile([128, 2, T], BF16, tag="qT", name="qT_sb")
    g.kT_sb = singles.tile([128, 2, T], BF16, tag="kT", name="kT_sb")
    # vaug[:, kc, h, :]: per k-chunk, per head: cols 0:64 = v + bias, cols
    # 64:96 zero, col 96 = 1.0 (the O^T matmul emits the softmax denominator
    # in psum row 96; engine partition reads must be 32-aligned).  Padded to
    # 128 cols (97:128 zero) so the O ldweights gets Fast Weight Load (FWL
    # needs NumWeights==128), halving its weight-load time.
    g.vaug = singles.tile([128, NKC, 4, 128], BF16, tag="vaug", name="vaug")
    nc.vector.memset(g.vaug[:, :, :, D:128], 0.0)
    nc.vector.memset(g.vaug[:, :, :, 96], 1.0)
    g.yT_sb = singles.tile([128, 2, T], IN_DT, tag="yT", name="yT_sb")

    # qkv for t < 1024
    _emit_qkv_tblock(nc, g, 0, xts=xts0)
    _emit_qkv_tblock(nc, g, 1)
    # attention for q < 1024 with qkv t>=1024 braided between heads: the
    # dense K=128 qkv chains keep the PE array warm through the exp-paced
    # attention stream, split into ~4.3us qk / ~5us v bursts (a whole 8.3us
    # tblock burst reliably trips the activity throttler into a 3.4us
    # half-clock window).  Heads 2,3 first so c_proj (chunk-1-first) can
    # start before heads 0,1 finish.  x DMAs issue one slot early so the
    # transfer hides under the preceding attention block.
    # blocks followed by qk braids defer their normalize past the braid's
    # DVE bias-adds: those adds gate the next block's first S matmuls (pool_x
    # bank reuse), and the normalize's reciprocal/multiply would otherwise
    # sit ahead of them in the DVE FIFO (~2us exposed stall)
    xts2 = _dma_xts(nc, g, 2)
    norm2q = _emit_attention_block(nc, g, 2, 0, 1024, defer_norm=True)
    _emit_qkv_qk(nc, g, 2, xts2)
    norm2q()
    _emit_attention_block(nc, g, 3, 0, 1024)
    _emit_qkv_v(nc, g, 2, xts2)
    norm0q = _emit_attention_block(nc, g, 0, 0, 1024, defer_norm=True)
    # c_proj weights (needed from proj phase on; DMA fits mid-kernel)
    xts3 = _dma_xts(nc, g, 3)
    g.wp_sb = singles.tile([128, 2, C], IN_DT, name="wp_sb")
    nc.sync.dma_start(
        out=g.wp_sb, in_=ins["wproj"].ap().rearrange("(i p) n -> p i n", p=128)
    )
    _emit_qkv_qk(nc, g, 3, xts3)
    norm0q()
    _emit_attention_block(nc, g, 1, 0, 1024)
    _emit_qkv_v(nc, g, 3, xts3)

    # attention for q >= 1024 with proj t<1024 tiles braided between heads
    # (the PE is strict FIFO, so filler inside a block would delay the
    # exp-paced S/O chain, and a proj tile placed right after the m0 block
    # that produces its yT stalls on that block's normalize-chain latency;
    # between-block bursts one block later are the right placement)
    norm2 = _emit_attention_block(nc, g, 2, 1024, 1024, defer_norm=True)
    _emit_proj_tile(nc, g, 0, out)
    norm2()
    for i in (1, 2):
        _emit_proj_tile(nc, g, i, out)
    norm3 = _emit_attention_block(nc, g, 3, 1024, 1024, defer_norm=True)
    _emit_proj_tile(nc, g, 3, out)
    norm3()
    for i in (4, 5):
        _emit_proj_tile(nc, g, i, out)
    norm0 = _emit_attention_block(nc, g, 0, 1024, 1024, defer_norm=True)
    _emit_proj_tile(nc, g, 6, out)
    norm0()
    _emit_proj_tile(nc, g, 7, out)
    # last head split into 768+256-wide sub-blocks: proj tiles 8..13 only
    # need q-cols 1024:1792, so they run between the sub-blocks and just 2
    # proj tiles trail the final (short, DMA-free) normalize
    _emit_attention_block(nc, g, 1, 1024, 768)
    # tiles 8,9 paired: their chunk-0 reads head-1 yT that the 768-block just
    # produced; leading with both tiles' chunk-1 plus a short junk fill
    # hides the normalize latency without an idle (HAM-tripping) window
    _emit_proj_pair(nc, g, 8, 9, out, fill_n=3)
    for i in (10, 11, 12, 13):
        _emit_proj_tile(nc, g, i, out)
    _emit_attention_tail256(nc, g, 1, 1792)
    _emit_proj_pair(nc, g, 14, 15, out)


@functools.lru_cache(maxsize=1)
def _program():
    nc = bacc.Bacc("TRN2", target_bir_lowering=False, debug=False)
    shapes = {
        "xT": ([C, T], IN_DT),
        "wq": ([C, DLOC], IN_DT),
        "wk": ([C, DLOC], IN_DT),
        "wv": ([C, DLOC], IN_DT),
        "bq": ([2, 128], F32),
        "bk": ([2, 128], F32),
        "bv": ([1, DLOC], F32),
        "wproj": ([DLOC, C], IN_DT),
        "mneg": ([128, NKC], F32),
        "tri": ([128, 128], BF16),
    }
    ins = {
        name: nc.dram_tensor(name, shape, dt_, kind="ExternalInput")
        for name, (shape, dt_) in shapes.items()
    }
    out = nc.dram_tensor("out", [T, C], BF16, kind="ExternalOutput")
    rn_dram = nc.dram_tensor("rn_scratch", [8, 1024], F32, kind="Internal")
    rn2_dram = nc.dram_tensor("rn2_scratch", [8, 1024], F32, kind="Internal")
    with tile.TileContext(nc) as tc, contextlib.ExitStack() as ctx:
        _build(ctx, nc, tc, ins, out, rn_dram, rn2_dram)
    nc.compile()
    return nc


def make_in_maps(x, attention_mask, W_attn, b_attn, W_proj, b_proj):
    import ml_dtypes

    in_np = ml_dtypes.bfloat16 if USE_BF16_INPUTS else np.float32
    x = np.ascontiguousarray(np.asarray(x, dtype=np.float32))
    attention_mask = np.asarray(attention_mask, dtype=np.float32)
    W_attn = np.asarray(W_attn, dtype=np.float32)
    b_attn = np.asarray(b_attn, dtype=np.float32)
    W_proj = np.asarray(W_proj, dtype=np.float32)

    tri = (np.arange(128)[None, :] >= np.arange(128)[:, None]).astype(np.float32)
    in_maps = []
    for c in range(NCORES):
        b = c // 4
        g = c % 4
        cols = slice(g * DLOC, (g + 1) * DLOC)
        xT = np.ascontiguousarray(x[b].T.astype(in_np))
        mneg = np.ascontiguousarray((attention_mask[b] * NEG).reshape(NKC, 128).T)
        in_maps.append(
            {
                "xT": xT,
                "wq": np.ascontiguousarray(W_attn[:, cols].astype(in_np)),
                "wk": np.ascontiguousarray(W_attn[:, C : 2 * C][:, cols].astype(in_np)),
                "wv": np.ascontiguousarray(
                    W_attn[:, 2 * C : 3 * C][:, cols].astype(in_np)
                ),
                "bq": np.ascontiguousarray(b_attn[cols].reshape(2, 128)),
                "bk": np.ascontiguousarray(b_attn[C : 2 * C][cols].reshape(2, 128)),
                "bv": np.ascontiguousarray(b_attn[2 * C : 3 * C][cols].reshape(1, DLOC)),
                "wproj": np.ascontiguousarray(
                    W_proj[g * DLOC : (g + 1) * DLOC, :].astype(in_np)
                ),
                "mneg": mneg,
                "tri": tri.astype(in_np),
            }
        )
    return in_maps


def kernel(x, attention_mask, W_attn, b_attn, W_proj, b_proj, _res_hook=None):
    in_maps = make_in_maps(x, attention_mask, W_attn, b_attn, W_proj, b_proj)
    nc = _program()
    res = bass_utils.run_bass_kernel_spmd(nc, in_maps, core_ids=list(range(NCORES)))
    if _res_hook is not None:
        _res_hook(res)
    b_proj = np.asarray(b_proj, dtype=np.float32)
    y = np.zeros((B, T, C), dtype=np.float32)
    for c in range(NCORES):
        y[c // 4] += np.asarray(res.results[c]["out"]).astype(np.float32)
    y += b_proj[None, None, :]
    return y

